# revision 25
# baseline (speedup 1.0000x reference)
"""Multi-head attention (B=2, S=2048, E=1024, H=16) on 8 Trainium2 NeuronCores.

Sharding: tensor-parallel over heads — core i owns heads (2i, 2i+1).

v3 structure (per core):
  Phase A  (per batch): q/k/v projections for its 2 heads, feature-major
           [128 = headA(64)|headB(64), tokens]; v PE-transposed to
           token-major vtm[128, chunk, 128] = [vA|vB]. All x chunks for
           both batches are prefetched up front on the ACT HWDGE queue so
           batch-1 loads never queue behind phase-BC traffic.
  Phase BC (per batch, per 512-q block):
           scores^T via ROW-TILED matmul pairs — headA contracts on PE
           rows 0:64, headB on rows 64:128 concurrently. One 2048-element
           Exp per (2 kc x 2 heads) quad straight out of one PSUM tile.
           AV COL-TILED: vA -> psum partitions 0:64, vB -> 64:128 of one
           [128, 512] accumulator (concurrent pair per k-chunk).
           Softmax denominators via M=1 ones-matmul waves, 4 concurrent
           col positions {0,32,64,96} = (head x kc-parity), accumulated
           over 8 waves; partials collected to partition 0 by one strided
           DMA, summed + approx-reciprocal on DVE, partition-broadcast,
           then a single [128, 512] multiply produces both heads' attn.
  AllToAll: FOUR bf16 collectives (one per half-batch, [8,128,128] each),
           fired as soon as each half-batch of attention is done; only
           the last is on the tail.
  Phase D  (per chunk of 128 tokens): output projection, interleaved
           under phase BC of the other batch; ga/out DMAs ride the ACT
           queue so they never wait behind phase-BC sync-queue traffic.

Output sharding: core c owns tokens {batch} x {half h} x
[c*128, (c+1)*128) within that half — gather_out() reassembles.
Matmuls bf16; fp32 PSUM accumulation throughout.
"""

import numpy as np
import ml_dtypes

import concourse.bass as bass
import concourse.mybir as mybir
import concourse.tile as tile
from concourse import bacc
from concourse import bass_utils
from concourse.masks import make_identity

F32 = mybir.dt.float32
BF16 = mybir.dt.bfloat16
N_CORES = 8
P = 128

# Full problem dims (hardcoded per the harness contract)
B_FULL, S_FULL, E, H, D = 2, 2048, 1024, 16, 64
HPC = H // N_CORES            # heads per core = 2
F = HPC * D                   # feature cols per core = 128
SCALE = D ** -0.5
DEBUG = False


def build_nc(B=B_FULL, S=S_FULL):
    CDT = BF16
    T = B * S                 # tokens
    KO = E // P               # 8 contraction chunks over embed
    TC = min(512, S)          # phase-A token chunk
    NTC = S // TC             # chunks per batch
    QB = min(512, S)          # q block
    NQB = S // QB             # q blocks per batch
    KC = S // P               # k chunks per batch
    QUAD = min(2, KC)         # k chunks per exp call (x2 heads = 2048 els)
    NQUAD = KC // QUAD
    HB = min(1024, S)         # tokens per a2a chunk (half-batch)
    NCH = T // HB             # a2a chunks
    TPC = HB // N_CORES       # tokens per core per chunk

    nc = bacc.Bacc("TRN2", target_bir_lowering=False, debug=False,
                   num_devices=N_CORES)

    xT = nc.dram_tensor("xT", [E, T], CDT, kind="ExternalInput").ap()
    wq = nc.dram_tensor("wq", [E, F], CDT, kind="ExternalInput").ap()
    wk = nc.dram_tensor("wk", [E, F], CDT, kind="ExternalInput").ap()
    wv = nc.dram_tensor("wv", [E, F], CDT, kind="ExternalInput").ap()
    bq = nc.dram_tensor("bq", [F, 1], F32, kind="ExternalInput").ap()
    bk = nc.dram_tensor("bk", [F, 1], F32, kind="ExternalInput").ap()
    bv = nc.dram_tensor("bv", [F, 1], F32, kind="ExternalInput").ap()
    ow = nc.dram_tensor("ow", [E, E], CDT, kind="ExternalInput").ap()
    ob = nc.dram_tensor("ob", [1, E], F32, kind="ExternalInput").ap()
    # rows = NCH chunks x TPC tokens: chunk ch = (batch, half),
    # token j of chunk ch = global (batch, (ch%CPB)*HB + core*TPC + j)
    out = nc.dram_tensor("out", [NCH * TPC, E], F32, kind="ExternalOutput").ap()
    dbg = {}
    if DEBUG:
        dbg["pv"] = nc.dram_tensor("dbg_pv", [P, QB], F32,
                                   kind="ExternalOutput").ap()
        dbg["dcp"] = nc.dram_tensor("dbg_dcp", [97, QB], F32,
                                    kind="ExternalOutput").ap()
        dbg["rp0"] = nc.dram_tensor("dbg_rp0", [1, 2 * QB], F32,
                                    kind="ExternalOutput").ap()
        dbg["db"] = nc.dram_tensor("dbg_db", [P, QB], F32,
                                   kind="ExternalOutput").ap()

    Exp = mybir.ActivationFunctionType.Exp

    with tile.TileContext(nc) as tc:
        with tc.tile_pool(name="persist", bufs=1) as persist, \
             tc.tile_pool(name="pAw", bufs=1) as pAw, \
             tc.tile_pool(name="pXt", bufs=6) as pXt, \
             tc.tile_pool(name="pA", bufs=3) as pA, \
             tc.tile_pool(name="pE", bufs=2) as pE, \
             tc.tile_pool(name="pNr", bufs=2) as pNr, \
             tc.tile_pool(name="pN1", bufs=1) as pN1, \
             tc.tile_pool(name="pAt", bufs=3) as pAt, \
             tc.tile_pool(name="pD", bufs=1) as pD, \
             tc.tile_pool(name="pDg", bufs=2) as pDg, \
             tc.tile_pool(name="pDo", bufs=2) as pDo, \
             tc.tile_pool(name="psA", bufs=2, space="PSUM") as psA, \
             tc.tile_pool(name="psS", bufs=1, space="PSUM") as psS, \
             tc.tile_pool(name="psAV", bufs=2, space="PSUM") as psAV, \
             tc.tile_pool(name="dramp", bufs=1, space="DRAM") as dramp:
            bq_sb = persist.tile([P, 1], F32)
            bk_sb = persist.tile([P, 1], F32)
            bv_sb = persist.tile([P, 1], F32)
            nc.sync.dma_start(bq_sb, bq)
            nc.sync.dma_start(bk_sb, bk)
            nc.sync.dma_start(bv_sb, bv)
            wq_sb = pAw.tile([P, KO, F], CDT)
            wk_sb = pAw.tile([P, KO, F], CDT)
            wv_sb = pAw.tile([P, KO, F], CDT)
            ow_sb = pD.tile([P, KO, E], CDT)
            xTr = xT.rearrange("(ko p) t -> p ko t", p=P)
            nc.sync.dma_start(wq_sb, wq.rearrange("(ko p) f -> p ko f", p=P))
            # x chunks for BOTH batches prefetched on the ACT HWDGE queue
            xts = {}
            for b in range(B):
                for tcx in range(NTC):
                    t0 = b * S + tcx * TC
                    xt = pXt.tile([P, KO, TC], CDT, tag="xt")
                    nc.scalar.dma_start(xt, xTr[:, :, t0:t0 + TC])
                    xts[(b, tcx)] = xt
            nc.sync.dma_start(wk_sb, wk.rearrange("(ko p) f -> p ko f", p=P))
            nc.sync.dma_start(wv_sb, wv.rearrange("(ko p) f -> p ko f", p=P))

            ident = persist.tile([P, P], CDT)
            make_identity(nc, ident)
            ob_row = persist.tile([1, E], F32)
            nc.sync.dma_start(ob_row, ob)
            obb = persist.tile([P, E], F32)
            nc.gpsimd.partition_broadcast(obb, ob_row)
            onesK = persist.tile([P, 1], CDT)   # denominator-wave weights
            nc.vector.memset(onesK, 1.0)

            qfm = persist.tile([P, T], CDT)     # q^T  rows 0:64 A, 64:128 B
            kfm = persist.tile([P, T], CDT)     # k^T  same split
            vtm = persist.tile([P, T // P, P], CDT)  # v token-major [vA|vB]

            a2a_in = [dramp.tile([N_CORES, P, TPC], CDT, name=f"a2a_in{ch}")
                      for ch in range(NCH)]
            a2a_out = [dramp.tile([N_CORES, P, TPC], CDT, name=f"a2a_out{ch}")
                       for ch in range(NCH)]

            def phase_a(b):
                for tcx in range(NTC):
                    t0 = b * S + tcx * TC
                    xt = xts[(b, tcx)]
                    ps = psA.tile([P, TC], F32, tag="ps")
                    for ko in range(KO):
                        nc.tensor.matmul(ps, lhsT=wq_sb[:, ko], rhs=xt[:, ko],
                                         start=(ko == 0), stop=(ko == KO - 1))
                    nc.vector.tensor_scalar_add(qfm[:, t0:t0 + TC], ps, bq_sb)
                    ps = psA.tile([P, TC], F32, tag="ps")
                    for ko in range(KO):
                        nc.tensor.matmul(ps, lhsT=wk_sb[:, ko], rhs=xt[:, ko],
                                         start=(ko == 0), stop=(ko == KO - 1))
                    nc.vector.tensor_scalar_add(kfm[:, t0:t0 + TC], ps, bk_sb)
                    ps = psA.tile([P, TC], F32, tag="ps")
                    for ko in range(KO):
                        nc.tensor.matmul(ps, lhsT=wv_sb[:, ko], rhs=xt[:, ko],
                                         start=(ko == 0), stop=(ko == KO - 1))
                    vfm = pA.tile([P, TC], CDT, tag="vfm")
                    nc.vector.tensor_scalar_add(vfm, ps, bv_sb)
                    for sub in range(TC // P):
                        pst = psA.tile([P, P], CDT, tag="ps")
                        nc.tensor.transpose(pst, vfm[:, sub * P:(sub + 1) * P],
                                            ident)
                        nc.vector.tensor_copy(vtm[:, (t0 + sub * P) // P], pst)

            def phase_bc(b, qb):
                q0 = b * S + qb * QB
                # e: [feat(kpos), head, kc, q] — head-major so one Exp
                # covers both heads' quad straight out of one psum tile
                e = pE.tile([P, 2, KC, QB], CDT, tag="e")
                for quad in range(NQUAD):
                    sS = psS.tile([P, 2, QUAD, QB], F32, tag="sS")
                    for j in range(QUAD):
                        k0 = b * S + (quad * QUAD + j) * P
                        # row-tiled pair: A on PE rows 0:64, B on 64:128
                        nc.tensor.matmul(
                            sS[:, 0, j], lhsT=kfm[0:64, k0:k0 + P],
                            rhs=qfm[0:64, q0:q0 + QB], start=True, stop=True)
                        nc.tensor.matmul(
                            sS[:, 1, j], lhsT=kfm[64:128, k0:k0 + P],
                            rhs=qfm[64:128, q0:q0 + QB], start=True, stop=True)
                    g0 = quad * QUAD
                    nc.scalar.activation(e[:, :, g0:g0 + QUAD], sS, Exp,
                                         scale=SCALE)
                # AV col-tiled: vA -> partitions 0:64, vB -> 64:128
                pv = psAV.tile([P, QB], F32, tag="av")
                for kc in range(KC):
                    c = (b * S) // P + kc
                    nc.tensor.matmul(pv[0:64], lhsT=vtm[:, c, 0:64],
                                     rhs=e[:, 0, kc],
                                     start=(kc == 0), stop=(kc == KC - 1),
                                     skip_group_check=True)
                    nc.tensor.matmul(pv[64:128], lhsT=vtm[:, c, 64:128],
                                     rhs=e[:, 1, kc],
                                     start=(kc == 0), stop=(kc == KC - 1),
                                     skip_group_check=True)
                # denominator waves: M=1 ones-matmuls, 4 concurrent col
                # positions = (head, kc-parity), accumulated over KC/2 waves
                pd_ = psAV.tile([P, QB], F32, tag="av")
                for w in range(KC // 2):
                    for h in range(2):
                        nc.tensor.matmul(pd_[0 + 32 * h:1 + 32 * h],
                                         lhsT=onesK, rhs=e[:, h, 2 * w],
                                         start=(w == 0), stop=(w == KC // 2 - 1),
                                         tile_position=(0, 32 * h),
                                         skip_group_check=True)
                        nc.tensor.matmul(pd_[64 + 32 * h:65 + 32 * h],
                                         lhsT=onesK, rhs=e[:, h, 2 * w + 1],
                                         start=(w == 0), stop=(w == KC // 2 - 1),
                                         tile_position=(0, 64 + 32 * h),
                                         skip_group_check=True)
                # reduce the 4 partials (even+odd kc) to partition 0, recip
                dcp = pN1.tile([97, QB], F32, tag="dcp")
                nc.vector.tensor_copy(dcp[0:1], pd_[0:1])
                nc.vector.tensor_copy(dcp[32:33], pd_[32:33])
                nc.vector.tensor_copy(dcp[64:65], pd_[64:65])
                nc.vector.tensor_copy(dcp[96:97], pd_[96:97])
                dn4 = pN1.tile([1, 4, QB], F32, tag="dn4")
                nc.sync.dma_start(dn4, dcp[0:97:32])
                den2 = pN1.tile([1, 2, QB], F32, tag="den2")
                nc.vector.tensor_add(den2, dn4[:, 0:2], dn4[:, 2:4])
                rp0 = pN1.tile([1, 2, QB], F32, tag="rp0")
                nc.vector.reciprocal_approx_fast(rp0, den2)
                db = pNr.tile([P, 2, QB], F32, tag="db")
                nc.gpsimd.partition_broadcast(db, rp0)
                attn = pAt.tile([P, QB], CDT, tag="attn")
                nc.vector.tensor_mul(attn[0:64], pv[0:64], db[0:64, 0])
                nc.vector.tensor_mul(attn[64:128], pv[64:128], db[64:128, 1])
                if DEBUG and b == 0 and qb == 0:
                    pv_sb = pNr.tile([P, QB], F32, tag="dbgpv")
                    nc.vector.tensor_copy(pv_sb, pv)
                    nc.sync.dma_start(dbg["pv"], pv_sb)
                    nc.sync.dma_start(dbg["dcp"], dcp)
                    nc.sync.dma_start(dbg["rp0"], rp0.rearrange("p a b -> p (a b)"))
                    nc.sync.dma_start(dbg["db"], db[:, 1])
                # scatter into the a2a chunk for this half-batch
                ch = b * (NCH // B) + (qb * QB) // HB
                d0 = ((qb * QB) % HB) // TPC
                for i in range(QB // TPC):
                    nc.sync.dma_start(
                        a2a_in[ch][d0 + i],
                        attn[:, i * TPC:(i + 1) * TPC])

            def send_a2a(ch):
                nc.gpsimd.collective_compute(
                    "AllToAll", mybir.AluOpType.bypass,
                    replica_groups=[list(range(N_CORES))],
                    ins=[a2a_in[ch].opt()], outs=[a2a_out[ch].opt()])

            def phase_d(ch):
                ga = pDg.tile([P, N_CORES, TPC], CDT, tag="ga")
                nc.scalar.dma_start(ga, a2a_out[ch].rearrange("c p t -> p c t"))
                for n2 in range(E // 512):
                    pso = psA.tile([P, 512], F32, tag="ps")
                    for r in range(N_CORES):
                        nc.tensor.matmul(
                            pso[0:TPC], lhsT=ga[:, r],
                            rhs=ow_sb[:, r, n2 * 512:(n2 + 1) * 512],
                            start=(r == 0), stop=(r == N_CORES - 1))
                    osb = pDo.tile([TPC, 512], F32, tag="osb")
                    nc.vector.tensor_add(osb, pso[0:TPC],
                                         obb[0:TPC, n2 * 512:(n2 + 1) * 512])
                    nc.scalar.dma_start(
                        out[ch * TPC:(ch + 1) * TPC,
                            n2 * 512:(n2 + 1) * 512],
                        osb)

            CPB = NCH // B        # a2a chunks per batch
            phase_a(0)
            for qb in range(NQB):
                phase_bc(0, qb)
                if (qb + 1) % (HB // QB) == 0:
                    send_a2a((qb * QB) // HB)
            done_d = 0
            if B > 1:
                phase_a(1)
                nc.sync.dma_start(ow_sb, ow.rearrange("(r p) e -> p r e", p=P))
                phase_d(0)
                done_d = 1
                for qb in range(NQB):
                    phase_bc(1, qb)
                    if (qb + 1) % (HB // QB) == 0:
                        send_a2a(CPB + (qb * QB) // HB)
                    if qb == NQB - 2 and CPB > 1:
                        phase_d(1)
                        done_d = 2
            else:
                nc.sync.dma_start(ow_sb, ow.rearrange("(r p) e -> p r e", p=P))
            for ch in range(done_d, NCH):
                phase_d(ch)

    nc.compile()
    return nc


def make_in_maps(x, qkv_w, qkv_b, o_w, o_b, B=B_FULL, S=S_FULL):
    """Host-side sharding: full inputs -> per-core input dicts."""
    T = B * S
    idt = ml_dtypes.bfloat16
    x = np.asarray(x, dtype=np.float32)
    qkv_w = np.asarray(qkv_w, dtype=np.float32).astype(idt)
    qkv_b = np.asarray(qkv_b, dtype=np.float32)
    o_w = np.ascontiguousarray(np.asarray(o_w, dtype=np.float32).astype(idt))
    o_b = np.asarray(o_b, dtype=np.float32).reshape(1, E)
    xT = np.ascontiguousarray(x.reshape(T, E).T.astype(idt))
    in_maps = []
    for i in range(N_CORES):
        c0 = i * F
        in_maps.append({
            "xT": xT,
            "wq": np.ascontiguousarray(qkv_w[:, c0:c0 + F]),
            "wk": np.ascontiguousarray(qkv_w[:, E + c0:E + c0 + F]),
            "wv": np.ascontiguousarray(qkv_w[:, 2 * E + c0:2 * E + c0 + F]),
            "bq": np.ascontiguousarray(qkv_b[c0:c0 + F].reshape(F, 1)),
            "bk": np.ascontiguousarray(qkv_b[E + c0:E + c0 + F].reshape(F, 1)),
            "bv": np.ascontiguousarray(
                qkv_b[2 * E + c0:2 * E + c0 + F].reshape(F, 1)),
            "ow": o_w,
            "ob": o_b,
        })
    return in_maps


def gather_out(results, B=B_FULL, S=S_FULL):
    """Per-core [NCH*TPC, E] slices -> full [B, S, E]."""
    T = B * S
    HB = min(1024, S)
    NCH = T // HB
    TPC = HB // N_CORES
    CPB = NCH // B
    full = np.empty((B, S, E), dtype=np.float32)
    for c in range(N_CORES):
        r = results[c]["out"]
        for ch in range(NCH):
            b, h = ch // CPB, ch % CPB
            q0 = h * HB + c * TPC
            full[b, q0:q0 + TPC] = r[ch * TPC:(ch + 1) * TPC]
    return full


_NC_CACHE = {}


def _get_nc(B=B_FULL, S=S_FULL):
    key = (B, S)
    if key not in _NC_CACHE:
        _NC_CACHE[key] = build_nc(B, S)
    return _NC_CACHE[key]


def kernel(x, qkv_w, qkv_b, o_w, o_b):
    B, S, _ = np.asarray(x).shape
    nc = _get_nc(B, S)
    in_maps = make_in_maps(x, qkv_w, qkv_b, o_w, o_b, B, S)
    res = bass_utils.run_bass_kernel_spmd(
        nc, in_maps, core_ids=list(range(N_CORES)))
    return gather_out(res.results, B, S)


# revision 26
# speedup vs baseline: 1.2210x; 1.2210x over previous
"""Multi-head attention (B=2, S=2048, E=1024, H=16) on 8 Trainium2 NeuronCores.

Sharding: tensor-parallel over heads — core i owns heads (2i, 2i+1).

v3 structure (per core):
  Phase A  (per batch): q/k/v projections for its 2 heads, feature-major
           [128 = headA(64)|headB(64), tokens]; v PE-transposed to
           token-major vtm[128, chunk, 128] = [vA|vB]. All x chunks for
           both batches are prefetched up front on the ACT HWDGE queue so
           batch-1 loads never queue behind phase-BC traffic.
  Phase BC (per batch, per 512-q block):
           scores^T via ROW-TILED matmul pairs — headA contracts on PE
           rows 0:64, headB on rows 64:128 concurrently. One 2048-element
           Exp per (2 kc x 2 heads) quad straight out of one PSUM tile.
           AV COL-TILED: vA -> psum partitions 0:64, vB -> 64:128 of one
           [128, 512] accumulator (concurrent pair per k-chunk).
           Softmax denominators via M=1 ones-matmul waves, 4 concurrent
           col positions {0,32,64,96} = (head x kc-parity), accumulated
           over 8 waves; partials collected to partition 0 by one strided
           DMA, summed + approx-reciprocal on DVE, partition-broadcast,
           then a single [128, 512] multiply produces both heads' attn.
  AllToAll: FOUR bf16 collectives (one per half-batch, [8,128,128] each),
           fired as soon as each half-batch of attention is done; only
           the last is on the tail.
  Phase D  (per chunk of 128 tokens): output projection, interleaved
           under phase BC of the other batch; ga/out DMAs ride the ACT
           queue so they never wait behind phase-BC sync-queue traffic.

Output sharding: core c owns tokens {batch} x {half h} x
[c*128, (c+1)*128) within that half — gather_out() reassembles.
Matmuls bf16; fp32 PSUM accumulation throughout.
"""

import numpy as np
import ml_dtypes

import concourse.bass as bass
import concourse.mybir as mybir
import concourse.tile as tile
from concourse import bacc
from concourse import bass_utils
from concourse.masks import make_identity

F32 = mybir.dt.float32
BF16 = mybir.dt.bfloat16
N_CORES = 8
P = 128

# Full problem dims (hardcoded per the harness contract)
B_FULL, S_FULL, E, H, D = 2, 2048, 1024, 16, 64
HPC = H // N_CORES            # heads per core = 2
F = HPC * D                   # feature cols per core = 128
SCALE = D ** -0.5
DEBUG = False


def build_nc(B=B_FULL, S=S_FULL):
    CDT = BF16
    T = B * S                 # tokens
    KO = E // P               # 8 contraction chunks over embed
    TC = min(512, S)          # phase-A token chunk
    NTC = S // TC             # chunks per batch
    QB = min(512, S)          # q block
    NQB = S // QB             # q blocks per batch
    KC = S // P               # k chunks per batch
    QUAD = min(2, KC)         # k chunks per exp call (x2 heads = 2048 els)
    NQUAD = KC // QUAD
    HB = min(1024, S)         # tokens per a2a chunk (half-batch)
    NCH = T // HB             # a2a chunks
    TPC = HB // N_CORES       # tokens per core per chunk

    nc = bacc.Bacc("TRN2", target_bir_lowering=False, debug=False,
                   num_devices=N_CORES)

    xT = nc.dram_tensor("xT", [E, T], CDT, kind="ExternalInput").ap()
    wq = nc.dram_tensor("wq", [E, F], CDT, kind="ExternalInput").ap()
    wk = nc.dram_tensor("wk", [E, F], CDT, kind="ExternalInput").ap()
    wv = nc.dram_tensor("wv", [E, F], CDT, kind="ExternalInput").ap()
    bq = nc.dram_tensor("bq", [F, 1], F32, kind="ExternalInput").ap()
    bk = nc.dram_tensor("bk", [F, 1], F32, kind="ExternalInput").ap()
    bv = nc.dram_tensor("bv", [F, 1], F32, kind="ExternalInput").ap()
    ow = nc.dram_tensor("ow", [E, E], CDT, kind="ExternalInput").ap()
    ob = nc.dram_tensor("ob", [1, E], F32, kind="ExternalInput").ap()
    # rows = NCH chunks x TPC tokens: chunk ch = (batch, half),
    # token j of chunk ch = global (batch, (ch%CPB)*HB + core*TPC + j)
    out = nc.dram_tensor("out", [NCH * TPC, E], F32, kind="ExternalOutput").ap()
    dbg = {}
    if DEBUG:
        dbg["pv"] = nc.dram_tensor("dbg_pv", [P, QB], F32,
                                   kind="ExternalOutput").ap()
        dbg["dcp"] = nc.dram_tensor("dbg_dcp", [97, QB], F32,
                                    kind="ExternalOutput").ap()
        dbg["rp0"] = nc.dram_tensor("dbg_rp0", [1, 2 * QB], F32,
                                    kind="ExternalOutput").ap()
        dbg["db"] = nc.dram_tensor("dbg_db", [P, QB], F32,
                                   kind="ExternalOutput").ap()

    Exp = mybir.ActivationFunctionType.Exp

    with tile.TileContext(nc) as tc:
        with tc.tile_pool(name="persist", bufs=1) as persist, \
             tc.tile_pool(name="pAw", bufs=1) as pAw, \
             tc.tile_pool(name="pXt", bufs=6) as pXt, \
             tc.tile_pool(name="pA", bufs=3) as pA, \
             tc.tile_pool(name="pE", bufs=2) as pE, \
             tc.tile_pool(name="pNr", bufs=2) as pNr, \
             tc.tile_pool(name="pN1", bufs=1) as pN1, \
             tc.tile_pool(name="pAt", bufs=3) as pAt, \
             tc.tile_pool(name="pD", bufs=1) as pD, \
             tc.tile_pool(name="pDg", bufs=2) as pDg, \
             tc.tile_pool(name="pDo", bufs=2) as pDo, \
             tc.tile_pool(name="psA", bufs=2, space="PSUM") as psA, \
             tc.tile_pool(name="psS", bufs=1, space="PSUM") as psS, \
             tc.tile_pool(name="psAV", bufs=2, space="PSUM") as psAV, \
             tc.tile_pool(name="dramp", bufs=1, space="DRAM") as dramp:
            bq_sb = persist.tile([P, 1], F32)
            bk_sb = persist.tile([P, 1], F32)
            bv_sb = persist.tile([P, 1], F32)
            nc.sync.dma_start(bq_sb, bq)
            nc.sync.dma_start(bk_sb, bk)
            nc.sync.dma_start(bv_sb, bv)
            wq_sb = pAw.tile([P, KO, F], CDT)
            wk_sb = pAw.tile([P, KO, F], CDT)
            wv_sb = pAw.tile([P, KO, F], CDT)
            ow_sb = pD.tile([P, KO, E], CDT)
            xTr = xT.rearrange("(ko p) t -> p ko t", p=P)
            nc.sync.dma_start(wq_sb, wq.rearrange("(ko p) f -> p ko f", p=P))
            # x chunks for BOTH batches prefetched on the ACT HWDGE queue
            xts = {}
            nxt = 0
            for b in range(B):
                for tcx in range(NTC):
                    t0 = b * S + tcx * TC
                    xt = pXt.tile([P, KO, TC], CDT, tag="xt")
                    eng = nc.scalar if nxt < 6 else nc.gpsimd
                    eng.dma_start(xt, xTr[:, :, t0:t0 + TC])
                    nxt += 1
                    xts[(b, tcx)] = xt
            nc.sync.dma_start(wk_sb, wk.rearrange("(ko p) f -> p ko f", p=P))
            nc.sync.dma_start(wv_sb, wv.rearrange("(ko p) f -> p ko f", p=P))

            ident = persist.tile([P, P], CDT)
            make_identity(nc, ident)
            ob_row = persist.tile([1, E], F32)
            nc.sync.dma_start(ob_row, ob)
            obb = persist.tile([P, E], F32)
            nc.gpsimd.partition_broadcast(obb, ob_row)
            onesK = persist.tile([P, 1], CDT)   # denominator-wave weights
            nc.vector.memset(onesK, 1.0)

            qfm = persist.tile([P, T], CDT)     # q^T  rows 0:64 A, 64:128 B
            kfm = persist.tile([P, T], CDT)     # k^T  same split
            vtm = persist.tile([P, T // P, P], CDT)  # v token-major [vA|vB]

            a2a_in = [dramp.tile([N_CORES, P, TPC], CDT, name=f"a2a_in{ch}")
                      for ch in range(NCH)]
            a2a_out = [dramp.tile([N_CORES, P, TPC], CDT, name=f"a2a_out{ch}")
                       for ch in range(NCH)]

            def phase_a(b):
                for tcx in range(NTC):
                    t0 = b * S + tcx * TC
                    xt = xts[(b, tcx)]
                    ps = psA.tile([P, TC], F32, tag="ps")
                    for ko in range(KO):
                        nc.tensor.matmul(ps, lhsT=wq_sb[:, ko], rhs=xt[:, ko],
                                         start=(ko == 0), stop=(ko == KO - 1))
                    nc.vector.tensor_scalar_add(qfm[:, t0:t0 + TC], ps, bq_sb)
                    ps = psA.tile([P, TC], F32, tag="ps")
                    for ko in range(KO):
                        nc.tensor.matmul(ps, lhsT=wk_sb[:, ko], rhs=xt[:, ko],
                                         start=(ko == 0), stop=(ko == KO - 1))
                    nc.vector.tensor_scalar_add(kfm[:, t0:t0 + TC], ps, bk_sb)
                    ps = psA.tile([P, TC], F32, tag="ps")
                    for ko in range(KO):
                        nc.tensor.matmul(ps, lhsT=wv_sb[:, ko], rhs=xt[:, ko],
                                         start=(ko == 0), stop=(ko == KO - 1))
                    vfm = pA.tile([P, TC], CDT, tag="vfm")
                    nc.vector.tensor_scalar_add(vfm, ps, bv_sb)
                    for sub in range(TC // P):
                        pst = psA.tile([P, P], CDT, tag="ps")
                        nc.tensor.transpose(pst, vfm[:, sub * P:(sub + 1) * P],
                                            ident)
                        nc.vector.tensor_copy(vtm[:, (t0 + sub * P) // P], pst)

            def phase_bc(b, qb):
                q0 = b * S + qb * QB
                # e: [feat(kpos), head, kc, q] — head-major so one Exp
                # covers both heads' quad straight out of one psum tile
                e = pE.tile([P, 2, KC, QB], CDT, tag="e")
                for quad in range(NQUAD):
                    sS = psS.tile([P, 2, QUAD, QB], F32, tag="sS")
                    for j in range(QUAD):
                        k0 = b * S + (quad * QUAD + j) * P
                        # row-tiled pair: A on PE rows 0:64, B on 64:128
                        nc.tensor.matmul(
                            sS[:, 0, j], lhsT=kfm[0:64, k0:k0 + P],
                            rhs=qfm[0:64, q0:q0 + QB], start=True, stop=True)
                        nc.tensor.matmul(
                            sS[:, 1, j], lhsT=kfm[64:128, k0:k0 + P],
                            rhs=qfm[64:128, q0:q0 + QB], start=True, stop=True)
                    g0 = quad * QUAD
                    nc.scalar.activation(e[:, :, g0:g0 + QUAD], sS, Exp,
                                         scale=SCALE)
                # AV col-tiled: vA -> partitions 0:64, vB -> 64:128
                pv = psAV.tile([P, QB], F32, tag="av")
                for kc in range(KC):
                    c = (b * S) // P + kc
                    nc.tensor.matmul(pv[0:64], lhsT=vtm[:, c, 0:64],
                                     rhs=e[:, 0, kc],
                                     start=(kc == 0), stop=(kc == KC - 1),
                                     skip_group_check=True)
                    nc.tensor.matmul(pv[64:128], lhsT=vtm[:, c, 64:128],
                                     rhs=e[:, 1, kc],
                                     start=(kc == 0), stop=(kc == KC - 1),
                                     skip_group_check=True)
                # denominator waves: M=1 ones-matmuls, 4 concurrent col
                # positions = (head, kc-parity), accumulated over KC/2 waves
                pd_ = psAV.tile([P, QB], F32, tag="av")
                for w in range(KC // 2):
                    for h in range(2):
                        nc.tensor.matmul(pd_[0 + 32 * h:1 + 32 * h],
                                         lhsT=onesK, rhs=e[:, h, 2 * w],
                                         start=(w == 0), stop=(w == KC // 2 - 1),
                                         tile_position=(0, 32 * h),
                                         skip_group_check=True)
                        nc.tensor.matmul(pd_[64 + 32 * h:65 + 32 * h],
                                         lhsT=onesK, rhs=e[:, h, 2 * w + 1],
                                         start=(w == 0), stop=(w == KC // 2 - 1),
                                         tile_position=(0, 64 + 32 * h),
                                         skip_group_check=True)
                # reduce the 4 partials (even+odd kc) to partition 0, recip
                dcp = pN1.tile([97, QB], F32, tag="dcp")
                nc.vector.tensor_copy(dcp[0:1], pd_[0:1])
                nc.vector.tensor_copy(dcp[32:33], pd_[32:33])
                nc.vector.tensor_copy(dcp[64:65], pd_[64:65])
                nc.vector.tensor_copy(dcp[96:97], pd_[96:97])
                dn4 = pN1.tile([1, 4, QB], F32, tag="dn4")
                nc.sync.dma_start(dn4, dcp[0:97:32])
                den2 = pN1.tile([1, 2, QB], F32, tag="den2")
                nc.vector.tensor_add(den2, dn4[:, 0:2], dn4[:, 2:4])
                rp0 = pN1.tile([1, 2, QB], F32, tag="rp0")
                nc.vector.reciprocal_approx_fast(rp0, den2)
                db = pNr.tile([P, 2, QB], F32, tag="db")
                nc.gpsimd.partition_broadcast(db, rp0)
                attn = pAt.tile([P, QB], CDT, tag="attn")
                nc.vector.tensor_mul(attn[0:64], pv[0:64], db[0:64, 0])
                nc.vector.tensor_mul(attn[64:128], pv[64:128], db[64:128, 1])
                if DEBUG and b == 0 and qb == 0:
                    pv_sb = pNr.tile([P, QB], F32, tag="dbgpv")
                    nc.vector.tensor_copy(pv_sb, pv)
                    nc.sync.dma_start(dbg["pv"], pv_sb)
                    nc.sync.dma_start(dbg["dcp"], dcp)
                    nc.sync.dma_start(dbg["rp0"], rp0.rearrange("p a b -> p (a b)"))
                    nc.sync.dma_start(dbg["db"], db[:, 1])
                # scatter into the a2a chunk for this half-batch
                ch = b * (NCH // B) + (qb * QB) // HB
                d0 = ((qb * QB) % HB) // TPC
                for i in range(QB // TPC):
                    nc.sync.dma_start(
                        a2a_in[ch][d0 + i],
                        attn[:, i * TPC:(i + 1) * TPC])

            def send_a2a(ch):
                nc.gpsimd.collective_compute(
                    "AllToAll", mybir.AluOpType.bypass,
                    replica_groups=[list(range(N_CORES))],
                    ins=[a2a_in[ch].opt()], outs=[a2a_out[ch].opt()])

            def phase_d(ch):
                ga = pDg.tile([P, N_CORES, TPC], CDT, tag="ga")
                nc.sync.dma_start(ga, a2a_out[ch].rearrange("c p t -> p c t"))
                for n2 in range(E // 512):
                    pso = psA.tile([P, 512], F32, tag="ps")
                    for r in range(N_CORES):
                        nc.tensor.matmul(
                            pso[0:TPC], lhsT=ga[:, r],
                            rhs=ow_sb[:, r, n2 * 512:(n2 + 1) * 512],
                            start=(r == 0), stop=(r == N_CORES - 1))
                    osb = pDo.tile([TPC, 512], F32, tag="osb")
                    nc.vector.tensor_add(osb, pso[0:TPC],
                                         obb[0:TPC, n2 * 512:(n2 + 1) * 512])
                    nc.sync.dma_start(
                        out[ch * TPC:(ch + 1) * TPC,
                            n2 * 512:(n2 + 1) * 512],
                        osb)

            CPB = NCH // B        # a2a chunks per batch
            phase_a(0)
            for qb in range(NQB):
                phase_bc(0, qb)
                if (qb + 1) % (HB // QB) == 0:
                    send_a2a((qb * QB) // HB)
            done_d = 0
            if B > 1:
                phase_a(1)
                nc.sync.dma_start(ow_sb, ow.rearrange("(r p) e -> p r e", p=P))
                phase_d(0)
                done_d = 1
                for qb in range(NQB):
                    phase_bc(1, qb)
                    if (qb + 1) % (HB // QB) == 0:
                        send_a2a(CPB + (qb * QB) // HB)
                    if qb == NQB - 2 and CPB > 1:
                        phase_d(1)
                        done_d = 2
            else:
                nc.sync.dma_start(ow_sb, ow.rearrange("(r p) e -> p r e", p=P))
            for ch in range(done_d, NCH):
                phase_d(ch)

    nc.compile()
    return nc


def make_in_maps(x, qkv_w, qkv_b, o_w, o_b, B=B_FULL, S=S_FULL):
    """Host-side sharding: full inputs -> per-core input dicts."""
    T = B * S
    idt = ml_dtypes.bfloat16
    x = np.asarray(x, dtype=np.float32)
    qkv_w = np.asarray(qkv_w, dtype=np.float32).astype(idt)
    qkv_b = np.asarray(qkv_b, dtype=np.float32)
    o_w = np.ascontiguousarray(np.asarray(o_w, dtype=np.float32).astype(idt))
    o_b = np.asarray(o_b, dtype=np.float32).reshape(1, E)
    xT = np.ascontiguousarray(x.reshape(T, E).T.astype(idt))
    in_maps = []
    for i in range(N_CORES):
        c0 = i * F
        in_maps.append({
            "xT": xT,
            "wq": np.ascontiguousarray(qkv_w[:, c0:c0 + F]),
            "wk": np.ascontiguousarray(qkv_w[:, E + c0:E + c0 + F]),
            "wv": np.ascontiguousarray(qkv_w[:, 2 * E + c0:2 * E + c0 + F]),
            "bq": np.ascontiguousarray(qkv_b[c0:c0 + F].reshape(F, 1)),
            "bk": np.ascontiguousarray(qkv_b[E + c0:E + c0 + F].reshape(F, 1)),
            "bv": np.ascontiguousarray(
                qkv_b[2 * E + c0:2 * E + c0 + F].reshape(F, 1)),
            "ow": o_w,
            "ob": o_b,
        })
    return in_maps


def gather_out(results, B=B_FULL, S=S_FULL):
    """Per-core [NCH*TPC, E] slices -> full [B, S, E]."""
    T = B * S
    HB = min(1024, S)
    NCH = T // HB
    TPC = HB // N_CORES
    CPB = NCH // B
    full = np.empty((B, S, E), dtype=np.float32)
    for c in range(N_CORES):
        r = results[c]["out"]
        for ch in range(NCH):
            b, h = ch // CPB, ch % CPB
            q0 = h * HB + c * TPC
            full[b, q0:q0 + TPC] = r[ch * TPC:(ch + 1) * TPC]
    return full


_NC_CACHE = {}


def _get_nc(B=B_FULL, S=S_FULL):
    key = (B, S)
    if key not in _NC_CACHE:
        _NC_CACHE[key] = build_nc(B, S)
    return _NC_CACHE[key]


def kernel(x, qkv_w, qkv_b, o_w, o_b):
    B, S, _ = np.asarray(x).shape
    nc = _get_nc(B, S)
    in_maps = make_in_maps(x, qkv_w, qkv_b, o_w, o_b, B, S)
    res = bass_utils.run_bass_kernel_spmd(
        nc, in_maps, core_ids=list(range(N_CORES)))
    return gather_out(res.results, B, S)
